# revision 37
# baseline (speedup 1.0000x reference)
"""Causal self-attention (B=2, T=2048, D=1024, H=16, Dh=64) on 8 NeuronCores.

Sharding: tensor-parallel over heads. Core c owns heads {2c, 2c+1}:
  - QKV: computes q/k/v columns c*128:(c+1)*128 of each section.
      q,k are produced transposed (qT/kT: [128 qkv-cols, tokens]) via
      out = w3_slice.T @ x.T matmuls; v is produced in natural layout
      ([tokens, 128 v-cols]) via PE transposes of the vT chunks.
  - Attention: per (batch, q-chunk of 512): a per-k-tile software pipeline
      S^T = K_h @ Q_h.T (both heads packed via disjoint 64-row groups into
      one double-buffered [128, 2, 512] PSUM block), exp on ACT (|S*scale|
      <= ~6 so no max subtraction), causal mask via affine_select on
      diagonal tiles only (fill=0 after exp), then out^T accumulated as
      V'.T @ P^T where V' = [V | ones]: row 64 of the PSUM accumulator is
      the softmax denominator. Diagonal k-tiles are column-truncated
      (widths 512/384/256/256 - kept >= 256 so fp32r stays 1 cyc/row),
      skipping fully-masked q columns in S, exp and PV.
      PV(kt) is emitted one step behind S/exp(kt+1) so the PE never waits
      on the ACT exp and HAM stays at K=8/8.
  - Normalize: PV accumulators are evacuated to SBUF immediately (frees
      PSUM for the next q-chunk), denominator rows are partition-broadcast
      via a DRAM bounce (DMA-only), inverted with reciprocal_approx_fast
      on aligned SBUF tiles, and the per-head multiply runs on DVE.
  - Projection: partial out^T = w_proj_slice.T applied per 128-row slice;
      per-core partial [1024, 4096] outputs are summed on the host.
  - QKV chunks of batch 1 are interleaved with attention q-chunks of
      batch 0 so the PE has filler work during ACT-bound stretches.

All matmuls run in float32r (4-byte data, reduced-precision multiply,
1 cycle/row for moving dims >= 256 -- 4x faster than plain fp32).

TRN2 allows at most one sync-wait per instruction; bacc's
generate_event_semaphores pass splits multi-wait instructions, which is
why the program is built with bacc.Bacc and compiled before dispatch.
"""

import numpy as np

D_MODEL = 1024
B, T = 2, 2048
RC = 128  # per-core qkv columns per q/k/v section == per-core w_proj rows
M = B * T
N_CORES = 8

_prog_cache = {}
_last_results = None  # BassKernelResults of the most recent run (for profiling)


def build_program(Tb=T, use_vbias=False):
    from contextlib import ExitStack

    import concourse.bass as bass
    import concourse.tile as tile
    from concourse import bacc, mybir
    from concourse.tile import add_dep_helper

    f32 = mybir.dt.float32
    f32r = mybir.dt.float32r
    EXP = mybir.ActivationFunctionType.Exp
    MULT = mybir.AluOpType.mult
    IS_GE = mybir.AluOpType.is_ge

    Mb = B * Tb
    mc_per_b = Tb // 512  # x/m chunks of 512 tokens per batch
    mt_per_b = Tb // 128  # v tiles of 128 tokens per batch
    n_qc = Tb // 512      # query chunks per batch

    nc = bacc.Bacc("TRN2", target_bir_lowering=False, debug=False)
    xT = nc.dram_tensor("xT", [D_MODEL, Mb], f32r, kind="ExternalInput").ap()
    w3 = nc.dram_tensor("w3", [D_MODEL, 3 * RC], f32r, kind="ExternalInput").ap()
    wp = nc.dram_tensor("wp", [RC, D_MODEL], f32r, kind="ExternalInput").ap()
    bqk = nc.dram_tensor("bqk", [RC, 2], f32, kind="ExternalInput").ap()
    ident = nc.dram_tensor("ident", [128, 128], f32r, kind="ExternalInput").ap()
    bv = None
    if use_vbias:
        bv = nc.dram_tensor("bv", [RC, 1], f32, kind="ExternalInput").ap()
    out_d = nc.dram_tensor("out", [D_MODEL, Mb], f32, kind="ExternalOutput").ap()
    scr_d = nc.dram_tensor("scr", [2 * n_qc, 2, 512], f32).ap()  # recip bounce

    xT_r = xT.rearrange("(kt p) m -> p kt m", p=128)  # [128, 8, Mb]
    w3_r = w3.rearrange("(kt p) n -> p kt n", p=128)  # [128, 8, 384]

    with tile.TileContext(nc) as tc:
        with ExitStack() as ctx:
            singles = ctx.enter_context(tc.tile_pool(name="singles", bufs=1))
            xpool = ctx.enter_context(tc.tile_pool(name="xpool", bufs=3))
            ptp = ctx.enter_context(tc.tile_pool(name="ptp", bufs=4))
            pt2p = ctx.enter_context(tc.tile_pool(name="pt2p", bufs=3))
            rcp = ctx.enter_context(tc.tile_pool(name="rcp", bufs=2))
            rbp = ctx.enter_context(tc.tile_pool(name="rbp", bufs=2))
            pvcp = ctx.enter_context(tc.tile_pool(name="pvcp", bufs=6))
            vtp = ctx.enter_context(tc.tile_pool(name="vtp", bufs=2))
            obp = ctx.enter_context(tc.tile_pool(name="obp", bufs=6))
            ps_a = ctx.enter_context(tc.tile_pool(name="ps_a", bufs=2, space="PSUM"))
            ps_s = ctx.enter_context(tc.tile_pool(name="ps_s", bufs=2, space="PSUM"))
            ps_pv = ctx.enter_context(tc.tile_pool(name="ps_pv", bufs=2, space="PSUM"))

            # identity first (tiny), then PE warmup matmuls so the HAM clock
            # gate is released by the time the first x chunk lands
            id_sb = singles.tile([128, 128], f32r, tag="ident")
            nc.sync.dma_start(id_sb, ident)
            # preload the exp activation table off the critical path
            actwu = singles.tile([1, 64], f32r, tag="actwu")
            nc.scalar.activation(actwu, id_sb[0:1, 0:64], EXP, scale=1.0)
            wu_ps = ps_a.tile([128, 512], f32, tag="mm")
            for _ in range(28):
                nc.tensor.matmul(wu_ps[:, 0:128], id_sb, id_sb,
                                 start=True, stop=True)

            # x chunks prefetched up front on the SP HWDGE ring; weights go
            # through the ACT HWDGE ring so the two streams don't serialize
            x_tiles = []
            for mc in range(B * mc_per_b):
                x_sb = xpool.tile([128, 8, 512], f32r, tag="x")
                nc.sync.dma_start(x_sb, xT_r[:, :, mc * 512:(mc + 1) * 512])
                x_tiles.append(x_sb)

            w3_sb = singles.tile([128, 8, 3 * RC], f32r, tag="w3")
            nc.scalar.dma_start(w3_sb, w3_r)
            wp_sb = singles.tile([128, D_MODEL], f32r, tag="wp")
            nc.scalar.dma_start(wp_sb, wp)
            bqk_sb = singles.tile([RC, 2], f32, tag="bqk")
            nc.scalar.dma_start(bqk_sb, bqk)
            bv_sb = None
            if use_vbias:
                bv_sb = singles.tile([RC, 1], f32, tag="bv")
                nc.scalar.dma_start(bv_sb, bv)

            qT, kT, vb, aT = {}, {}, {}, {}
            for b in range(B):
                qT[b] = singles.tile([128, Tb], f32r, tag=f"qT{b}", name=f"qT{b}")
                kT[b] = singles.tile([128, Tb], f32r, tag=f"kT{b}", name=f"kT{b}")
                vb[b] = singles.tile([128, mt_per_b, 130], f32r, tag=f"vb{b}",
                                     name=f"vb{b}")
                aT[b] = singles.tile([128, Tb], f32r, tag=f"aT{b}", name=f"aT{b}")
                # ones columns for the softmax-denominator rows of PV
                nc.vector.memset(vb[b][:, :, 64:65].bitcast(f32), 1.0)
                nc.vector.memset(vb[b][:, :, 129:130].bitcast(f32), 1.0)

            tails = {}

            def emit_qkv_chunk(b, mci):
                mc = b * mc_per_b + mci
                x_sb = x_tiles[mc]
                # qT / kT / vT: out[qkvcol, m] accumulated over 8 k-tiles
                vTs = None
                for nt in range(3):
                    ps = ps_a.tile([128, 512], f32, tag="mm")
                    for kt in range(8):
                        nc.tensor.matmul(
                            ps,
                            w3_sb[:, kt, nt * RC:(nt + 1) * RC],
                            x_sb[:, kt, :],
                            start=(kt == 0), stop=(kt == 7),
                        )
                    if nt < 2:
                        # psum->sbuf move fused with the bias add, on ACT
                        # (identity shares the exp table -> no table loads)
                        dest = qT[b] if nt == 0 else kT[b]
                        nc.scalar.add(
                            dest[:, mci * 512:(mci + 1) * 512], ps,
                            bqk_sb[:, nt:nt + 1],
                        )
                    else:
                        vTs = vtp.tile([128, 512], f32r, tag="vT")
                        nc.vector.tensor_copy(vTs, ps)
                # transpose vT chunks into natural [tokens, vcol] layout
                tp = ps_a.tile([128, 512], f32, tag="mm")
                for ms in range(4):
                    nc.tensor.transpose(
                        tp[:, ms * 128:(ms + 1) * 128].bitcast(f32r),
                        vTs[:, ms * 128:(ms + 1) * 128],
                        id_sb,
                    )
                for ms in range(4):
                    mt = mci * 4 + ms
                    # one strided copy per 128-token tile: psum cols
                    # [0:64|64:128] -> vb cols [0:64|65:129]
                    sl = tp[:, ms * 128:(ms + 1) * 128].bitcast(f32r)
                    dsl = vb[b][:, mt, 0:129]
                    dst = bass.AP(tensor=dsl.tensor, offset=dsl.offset,
                                  ap=[dsl.ap[0], [65, 2], [1, 64]])
                    src = bass.AP(tensor=sl.tensor, offset=sl.offset,
                                  ap=[sl.ap[0], [64, 2], [1, 64]])
                    nc.vector.tensor_copy(dst, src)

            def emit_attn_qc(b, qc):
                nkt = (qc + 1) * 4
                pvs = (
                    ps_pv.tile([65, 512], f32, tag="pv", name="pv0"),
                    ps_pv.tile([65, 512], f32, tag="pv", name="pv1"),
                )
                # software pipeline: S/exp(kt) runs one step ahead of PV(kt)
                # so the PE never stalls on the ACT exp
                stage = []  # (kt, off, psrc)
                for kt in range(nkt + 1):
                    if kt < nkt:
                        diag = kt >= 4 * qc
                        off = min((kt - 4 * qc) * 128, 256) if diag else 0
                        w = 512 - off
                        s = ps_s.tile([128, 2, 512], f32, tag="s")
                        for h in (0, 1):
                            nc.tensor.matmul(
                                s[:, h, off:512],
                                kT[b][h * 64:(h + 1) * 64,
                                      kt * 128:(kt + 1) * 128],
                                qT[b][h * 64:(h + 1) * 64,
                                      qc * 512 + off:(qc + 1) * 512],
                                start=True, stop=True,
                            )
                        pt = ptp.tile([128, 2, 512], f32r, tag="pt")
                        if off == 0:
                            nc.scalar.activation(pt, s, EXP, scale=0.125)
                        else:
                            nc.scalar.activation(pt[:, :, off:512],
                                                 s[:, :, off:512],
                                                 EXP, scale=0.125)
                        if diag:  # causal mask, out-of-place, after exp
                            pt2 = pt2p.tile([128, 2, 512], f32r, tag="pt2")
                            for h in (0, 1):
                                nc.gpsimd.affine_select(
                                    pt2[:, h, off:512],
                                    pt[:, h, off:512],
                                    pattern=[[1, w]],
                                    compare_op=IS_GE,
                                    fill=0.0,
                                    base=qc * 512 + off - kt * 128,
                                    channel_multiplier=-1,
                                )
                            psrc = pt2
                        else:
                            psrc = pt
                        stage.append((kt, off, psrc))
                    if kt >= 1:
                        pkt, poff, ppsrc = stage[kt - 1]
                        for h in (0, 1):
                            nc.tensor.matmul(
                                pvs[h][:, poff:512],
                                vb[b][:, pkt, h * 65:(h + 1) * 65],
                                ppsrc[:, h, poff:512],
                                start=(pkt == 0), stop=(pkt == nkt - 1),
                            )
                # denominators: approx-reciprocal straight off PSUM row 64,
                # partition-broadcast with a K=2 selector matmul, multiply
                # fused into the PSUM->SBUF move
                # evacuate the PV accumulators to SBUF immediately (frees the
                # PSUM banks for the next q-chunk without waiting on the
                # normalize chain); row 64 is the softmax denominator, which
                # is bounced to DRAM here -- the read-back, reciprocal and
                # multiply are emitted a full iteration later (emit_norm) so
                # the bounce latency never blocks any engine FIFO
                pvcs = []
                d1s = []
                slot = b * n_qc + qc
                for h in (0, 1):
                    pvc = pvcp.tile([65, 512], f32, tag="pvc")
                    nc.vector.tensor_copy(pvc, pvs[h])
                    pvcs.append(pvc)
                    d1s.append(nc.sync.dma_start(scr_d[slot, h:h + 1, :],
                                                 pvc[64:65, :]))
                tails[(b, qc)] = (pvcs, d1s)

            def emit_norm(b, qc):
                pvcs, d1s = tails.pop((b, qc))
                slot = b * n_qc + qc
                # one partition-broadcast read for both heads' denominators
                a0 = scr_d[slot, 0:1, :]
                rb_t = rbp.tile([64, 2, 512], f32, tag="rb")
                d2 = nc.gpsimd.dma_start(rb_t, bass.AP(
                    tensor=a0.tensor, offset=a0.offset,
                    ap=[[0, 64], [1, 1024]]))
                for d1 in d1s:
                    add_dep_helper(d2.ins, d1.ins, reason="scr bounce RAW")
                # invert the broadcast denominators (SBUF, partition 0 -
                # custom DVE ops need aligned SBUF operands)
                rb_r = rcp.tile([64, 2, 512], f32, tag="rc")
                nc.vector.reciprocal_approx_fast(rb_r, rb_t)
                for h in (0, 1):
                    dst = aT[b][h * 64:(h + 1) * 64, qc * 512:(qc + 1) * 512]
                    nc.vector.tensor_tensor(dst, pvcs[h][0:64, :],
                                            rb_r[:, h, :], op=MULT)
                    if use_vbias:
                        nc.vector.tensor_scalar_add(
                            dst, dst, bv_sb[h * 64:(h + 1) * 64, 0:1]
                        )
            def emit_proj(b, qc):
                for nt in range(8):
                    ps = ps_a.tile([128, 512], f32, tag="mm")
                    nc.tensor.matmul(
                        ps,
                        wp_sb[:, nt * 128:(nt + 1) * 128],
                        aT[b][:, qc * 512:(qc + 1) * 512],
                        start=True, stop=True,
                    )
                    ob = obp.tile([128, 512], f32, tag="ob")
                    nc.vector.tensor_copy(ob, ps)
                    nc.sync.dma_start(
                        out_d[nt * 128:(nt + 1) * 128,
                              b * Tb + qc * 512: b * Tb + (qc + 1) * 512],
                        ob,
                    )

            # attention q-chunk qc only needs qkv chunks <= qc of its batch,
            # so attend as soon as each chunk's qT/kT/vb are ready; the
            # projection of chunk i-1 is emitted behind chunk i's whole
            # attention loop so the normalize chain's DMA-bounce latency
            # hides under attention matmuls instead of head-of-line-blocking
            # the PE queue
            prev = None
            deferred = None
            for b in range(B):
                for mc in range(mc_per_b):
                    last = (b == B - 1 and mc == mc_per_b - 1)
                    emit_qkv_chunk(b, mc)
                    emit_attn_qc(b, mc)
                    if prev is not None:
                        emit_norm(*prev)
                        if last:
                            # hold this projection back so it can fill the
                            # final normalize chain's DMA-bounce latency
                            deferred = prev
                        else:
                            emit_proj(*prev)
                    prev = (b, mc)
            emit_norm(*prev)
            emit_proj(*deferred)
            emit_proj(*prev)

    nc.compile()
    return nc


def make_in_maps(x, w_qkv, b_qkv, use_vbias):
    """Host-side shard prep. Returns per-core input maps (w_proj added later)."""
    Mx = x.shape[0] * x.shape[1]
    xT = np.ascontiguousarray(x.reshape(Mx, D_MODEL).T)
    in_maps = []
    for c in range(N_CORES):
        w3c = np.ascontiguousarray(
            np.concatenate(
                [w_qkv[:, s * D_MODEL + c * RC: s * D_MODEL + (c + 1) * RC]
                 for s in range(3)],
                axis=1,
            )
        )
        bqkc = np.ascontiguousarray(
            np.stack(
                [b_qkv[c * RC:(c + 1) * RC],
                 b_qkv[D_MODEL + c * RC: D_MODEL + (c + 1) * RC]],
                axis=1,
            )
        )
        im = {"xT": xT, "w3": w3c, "bqk": bqkc,
              "ident": np.eye(128, dtype=np.float32)}
        if use_vbias:
            im["bv"] = np.ascontiguousarray(
                b_qkv[2 * D_MODEL + c * RC: 2 * D_MODEL + (c + 1) * RC][:, None]
            )
        in_maps.append(im)
    return in_maps


def kernel(x, w_qkv, b_qkv, w_proj, b_proj):
    from concourse.bass_utils import run_bass_kernel_spmd

    x = np.asarray(x, dtype=np.float32)
    w_qkv = np.asarray(w_qkv, dtype=np.float32)
    b_qkv = np.asarray(b_qkv, dtype=np.float32)
    w_proj = np.asarray(w_proj, dtype=np.float32)
    b_proj = np.asarray(b_proj, dtype=np.float32)

    use_vbias = bool(np.any(b_qkv[2 * D_MODEL:]))
    key = (T, use_vbias)
    if key not in _prog_cache:
        _prog_cache[key] = build_program(T, use_vbias)
    nc = _prog_cache[key]

    in_maps = make_in_maps(x, w_qkv, b_qkv, use_vbias)
    for c in range(N_CORES):
        in_maps[c]["wp"] = np.ascontiguousarray(w_proj[c * RC:(c + 1) * RC, :])

    res = run_bass_kernel_spmd(nc, in_maps, core_ids=list(range(N_CORES)))
    global _last_results
    _last_results = res
    total = res.results[0]["out"].copy()
    for c in range(1, N_CORES):
        total += res.results[c]["out"]
    out = total.T.reshape(B, T, D_MODEL) + b_proj[None, None, :]
    return np.ascontiguousarray(out.astype(np.float32))


# revision 39
# speedup vs baseline: 1.0046x; 1.0046x over previous
"""Causal self-attention (B=2, T=2048, D=1024, H=16, Dh=64) on 8 NeuronCores.

Sharding: tensor-parallel over heads. Core c owns heads {2c, 2c+1}:
  - QKV: computes q/k/v columns c*128:(c+1)*128 of each section.
      q,k are produced transposed (qT/kT: [128 qkv-cols, tokens]) via
      out = w3_slice.T @ x.T matmuls; v is produced in natural layout
      ([tokens, 128 v-cols]) via PE transposes of the vT chunks.
  - Attention: per (batch, q-chunk of 512): a per-k-tile software pipeline
      S^T = K_h @ Q_h.T (both heads packed via disjoint 64-row groups into
      one double-buffered [128, 2, 512] PSUM block), exp on ACT (|S*scale|
      <= ~6 so no max subtraction), causal mask via affine_select on
      diagonal tiles only (fill=0 after exp), then out^T accumulated as
      V'.T @ P^T where V' = [V | ones]: row 64 of the PSUM accumulator is
      the softmax denominator. Diagonal k-tiles are column-truncated
      (widths 512/384/256/256 - kept >= 256 so fp32r stays 1 cyc/row),
      skipping fully-masked q columns in S, exp and PV.
      PV(kt) is emitted one step behind S/exp(kt+1) so the PE never waits
      on the ACT exp and HAM stays at K=8/8.
  - Normalize: PV accumulators are evacuated to SBUF immediately (frees
      PSUM for the next q-chunk), denominator rows are partition-broadcast
      via a DRAM bounce (DMA-only), inverted with reciprocal_approx_fast
      on aligned SBUF tiles, and the per-head multiply runs on DVE.
  - Projection: partial out^T = w_proj_slice.T applied per 128-row slice;
      per-core partial [1024, 4096] outputs are summed on the host.
  - QKV chunks of batch 1 are interleaved with attention q-chunks of
      batch 0 so the PE has filler work during ACT-bound stretches.

All matmuls run in float32r (4-byte data, reduced-precision multiply,
1 cycle/row for moving dims >= 256 -- 4x faster than plain fp32).

TRN2 allows at most one sync-wait per instruction; bacc's
generate_event_semaphores pass splits multi-wait instructions, which is
why the program is built with bacc.Bacc and compiled before dispatch.
"""

import numpy as np

D_MODEL = 1024
B, T = 2, 2048
RC = 128  # per-core qkv columns per q/k/v section == per-core w_proj rows
M = B * T
N_CORES = 8

_prog_cache = {}
_last_results = None  # BassKernelResults of the most recent run (for profiling)


def build_program(Tb=T, use_vbias=False):
    from contextlib import ExitStack

    import concourse.bass as bass
    import concourse.tile as tile
    from concourse import bacc, mybir
    from concourse.tile import add_dep_helper

    f32 = mybir.dt.float32
    f32r = mybir.dt.float32r
    EXP = mybir.ActivationFunctionType.Exp
    MULT = mybir.AluOpType.mult
    IS_GE = mybir.AluOpType.is_ge

    Mb = B * Tb
    mc_per_b = Tb // 512  # x/m chunks of 512 tokens per batch
    mt_per_b = Tb // 128  # v tiles of 128 tokens per batch
    n_qc = Tb // 512      # query chunks per batch

    nc = bacc.Bacc("TRN2", target_bir_lowering=False, debug=False)
    xT = nc.dram_tensor("xT", [D_MODEL, Mb], f32r, kind="ExternalInput").ap()
    w3 = nc.dram_tensor("w3", [D_MODEL, 3 * RC], f32r, kind="ExternalInput").ap()
    wp = nc.dram_tensor("wp", [RC, D_MODEL], f32r, kind="ExternalInput").ap()
    bqk = nc.dram_tensor("bqk", [RC, 2], f32, kind="ExternalInput").ap()
    ident = nc.dram_tensor("ident", [128, 128], f32r, kind="ExternalInput").ap()
    bv = None
    if use_vbias:
        bv = nc.dram_tensor("bv", [RC, 1], f32, kind="ExternalInput").ap()
    out_d = nc.dram_tensor("out", [D_MODEL, Mb], f32, kind="ExternalOutput").ap()
    scr_d = nc.dram_tensor("scr", [2 * n_qc, 2, 512], f32).ap()  # recip bounce

    xT_r = xT.rearrange("(kt p) m -> p kt m", p=128)  # [128, 8, Mb]
    w3_r = w3.rearrange("(kt p) n -> p kt n", p=128)  # [128, 8, 384]

    with tile.TileContext(nc) as tc:
        with ExitStack() as ctx:
            singles = ctx.enter_context(tc.tile_pool(name="singles", bufs=1))
            xpool = ctx.enter_context(tc.tile_pool(name="xpool", bufs=3))
            ptp = ctx.enter_context(tc.tile_pool(name="ptp", bufs=4))
            pt2p = ctx.enter_context(tc.tile_pool(name="pt2p", bufs=3))
            rcp = ctx.enter_context(tc.tile_pool(name="rcp", bufs=2))
            rbp = ctx.enter_context(tc.tile_pool(name="rbp", bufs=2))
            pvcp = ctx.enter_context(tc.tile_pool(name="pvcp", bufs=6))
            vtp = ctx.enter_context(tc.tile_pool(name="vtp", bufs=2))
            obp = ctx.enter_context(tc.tile_pool(name="obp", bufs=6))
            ps_a = ctx.enter_context(tc.tile_pool(name="ps_a", bufs=2, space="PSUM"))
            ps_s = ctx.enter_context(tc.tile_pool(name="ps_s", bufs=2, space="PSUM"))
            ps_pv = ctx.enter_context(tc.tile_pool(name="ps_pv", bufs=2, space="PSUM"))

            # identity first (tiny), then PE warmup matmuls so the HAM clock
            # gate is released by the time the first x chunk lands
            id_sb = singles.tile([128, 128], f32r, tag="ident")
            nc.sync.dma_start(id_sb, ident)
            # preload the exp activation table off the critical path
            actwu = singles.tile([1, 64], f32r, tag="actwu")
            nc.scalar.activation(actwu, id_sb[0:1, 0:64], EXP, scale=1.0)
            wu_ps = ps_a.tile([128, 512], f32, tag="mm")
            for _ in range(28):
                nc.tensor.matmul(wu_ps[:, 0:128], id_sb, id_sb,
                                 start=True, stop=True)

            # x chunks prefetched up front on the SP HWDGE ring; weights go
            # through the ACT HWDGE ring so the two streams don't serialize
            x_tiles = []
            for mc in range(B * mc_per_b):
                x_sb = xpool.tile([128, 8, 512], f32r, tag="x")
                nc.sync.dma_start(x_sb, xT_r[:, :, mc * 512:(mc + 1) * 512])
                x_tiles.append(x_sb)

            w3_sb = singles.tile([128, 8, 3 * RC], f32r, tag="w3")
            nc.scalar.dma_start(w3_sb, w3_r)
            wp_sb = singles.tile([128, D_MODEL], f32r, tag="wp")
            nc.scalar.dma_start(wp_sb, wp)
            bqk_sb = singles.tile([RC, 2], f32, tag="bqk")
            nc.scalar.dma_start(bqk_sb, bqk)
            bv_sb = None
            if use_vbias:
                bv_sb = singles.tile([RC, 1], f32, tag="bv")
                nc.scalar.dma_start(bv_sb, bv)

            qT, kT, vb, aT = {}, {}, {}, {}
            for b in range(B):
                qT[b] = singles.tile([128, Tb], f32r, tag=f"qT{b}", name=f"qT{b}")
                kT[b] = singles.tile([128, Tb], f32r, tag=f"kT{b}", name=f"kT{b}")
                vb[b] = singles.tile([128, mt_per_b, 130], f32r, tag=f"vb{b}",
                                     name=f"vb{b}")
                aT[b] = singles.tile([128, Tb], f32r, tag=f"aT{b}", name=f"aT{b}")
                # ones columns for the softmax-denominator rows of PV
                nc.vector.memset(vb[b][:, :, 64:65].bitcast(f32), 1.0)
                nc.vector.memset(vb[b][:, :, 129:130].bitcast(f32), 1.0)

            tails = {}

            def emit_qkv_chunk(b, mci):
                mc = b * mc_per_b + mci
                x_sb = x_tiles[mc]
                # qT / kT / vT: out[qkvcol, m] accumulated over 8 k-tiles
                vTs = None
                for nt in range(3):
                    ps = ps_a.tile([128, 512], f32, tag="mm")
                    for kt in range(8):
                        nc.tensor.matmul(
                            ps,
                            w3_sb[:, kt, nt * RC:(nt + 1) * RC],
                            x_sb[:, kt, :],
                            start=(kt == 0), stop=(kt == 7),
                        )
                    if nt < 2:
                        # psum->sbuf move fused with the bias add, on ACT
                        # (identity shares the exp table -> no table loads)
                        dest = qT[b] if nt == 0 else kT[b]
                        nc.scalar.add(
                            dest[:, mci * 512:(mci + 1) * 512], ps,
                            bqk_sb[:, nt:nt + 1],
                        )
                    else:
                        vTs = vtp.tile([128, 512], f32r, tag="vT")
                        nc.vector.tensor_copy(vTs, ps)
                # transpose vT chunks into natural [tokens, vcol] layout
                tp = ps_a.tile([128, 512], f32, tag="mm")
                for ms in range(4):
                    nc.tensor.transpose(
                        tp[:, ms * 128:(ms + 1) * 128].bitcast(f32r),
                        vTs[:, ms * 128:(ms + 1) * 128],
                        id_sb,
                    )
                for ms in range(4):
                    mt = mci * 4 + ms
                    # one strided copy per 128-token tile: psum cols
                    # [0:64|64:128] -> vb cols [0:64|65:129]
                    sl = tp[:, ms * 128:(ms + 1) * 128].bitcast(f32r)
                    dsl = vb[b][:, mt, 0:129]
                    dst = bass.AP(tensor=dsl.tensor, offset=dsl.offset,
                                  ap=[dsl.ap[0], [65, 2], [1, 64]])
                    src = bass.AP(tensor=sl.tensor, offset=sl.offset,
                                  ap=[sl.ap[0], [64, 2], [1, 64]])
                    nc.vector.tensor_copy(dst, src)

            def emit_attn_qc(b, qc):
                nkt = (qc + 1) * 4
                pvs = (
                    ps_pv.tile([65, 512], f32, tag="pv", name="pv0"),
                    ps_pv.tile([65, 512], f32, tag="pv", name="pv1"),
                )
                # software pipeline: S/exp(kt) runs one step ahead of PV(kt)
                # so the PE never stalls on the ACT exp
                stage = []  # (kt, off, psrc)
                for kt in range(nkt + 1):
                    if kt < nkt:
                        diag = kt >= 4 * qc
                        off = min((kt - 4 * qc) * 128, 256) if diag else 0
                        w = 512 - off
                        s = ps_s.tile([128, 2, 512], f32, tag="s")
                        for h in (0, 1):
                            nc.tensor.matmul(
                                s[:, h, off:512],
                                kT[b][h * 64:(h + 1) * 64,
                                      kt * 128:(kt + 1) * 128],
                                qT[b][h * 64:(h + 1) * 64,
                                      qc * 512 + off:(qc + 1) * 512],
                                start=True, stop=True,
                            )
                        pt = ptp.tile([128, 2, 512], f32r, tag="pt")
                        if off == 0:
                            nc.scalar.activation(pt, s, EXP, scale=0.125)
                        else:
                            # keep ACT access patterns contiguous per head
                            for h in (0, 1):
                                nc.scalar.activation(pt[:, h, off:512],
                                                     s[:, h, off:512],
                                                     EXP, scale=0.125)
                        if diag:  # causal mask, out-of-place, after exp
                            pt2 = pt2p.tile([128, 2, 512], f32r, tag="pt2")
                            for h in (0, 1):
                                nc.gpsimd.affine_select(
                                    pt2[:, h, off:512],
                                    pt[:, h, off:512],
                                    pattern=[[1, w]],
                                    compare_op=IS_GE,
                                    fill=0.0,
                                    base=qc * 512 + off - kt * 128,
                                    channel_multiplier=-1,
                                )
                            psrc = pt2
                        else:
                            psrc = pt
                        stage.append((kt, off, psrc))
                    if kt >= 1:
                        pkt, poff, ppsrc = stage[kt - 1]
                        for h in (0, 1):
                            nc.tensor.matmul(
                                pvs[h][:, poff:512],
                                vb[b][:, pkt, h * 65:(h + 1) * 65],
                                ppsrc[:, h, poff:512],
                                start=(pkt == 0), stop=(pkt == nkt - 1),
                            )
                # denominators: approx-reciprocal straight off PSUM row 64,
                # partition-broadcast with a K=2 selector matmul, multiply
                # fused into the PSUM->SBUF move
                # evacuate the PV accumulators to SBUF immediately (frees the
                # PSUM banks for the next q-chunk without waiting on the
                # normalize chain); row 64 is the softmax denominator, which
                # is bounced to DRAM here -- the read-back, reciprocal and
                # multiply are emitted a full iteration later (emit_norm) so
                # the bounce latency never blocks any engine FIFO
                pvcs = []
                d1s = []
                slot = b * n_qc + qc
                for h in (0, 1):
                    pvc = pvcp.tile([65, 512], f32, tag="pvc")
                    nc.vector.tensor_copy(pvc, pvs[h])
                    pvcs.append(pvc)
                    d1s.append(nc.sync.dma_start(scr_d[slot, h:h + 1, :],
                                                 pvc[64:65, :]))
                tails[(b, qc)] = (pvcs, d1s)

            def emit_norm(b, qc):
                pvcs, d1s = tails.pop((b, qc))
                slot = b * n_qc + qc
                # one partition-broadcast read for both heads' denominators
                a0 = scr_d[slot, 0:1, :]
                rb_t = rbp.tile([64, 2, 512], f32, tag="rb")
                d2 = nc.gpsimd.dma_start(rb_t, bass.AP(
                    tensor=a0.tensor, offset=a0.offset,
                    ap=[[0, 64], [1, 1024]]))
                for d1 in d1s:
                    add_dep_helper(d2.ins, d1.ins, reason="scr bounce RAW")
                # invert the broadcast denominators (SBUF, partition 0 -
                # custom DVE ops need aligned SBUF operands)
                rb_r = rcp.tile([64, 2, 512], f32, tag="rc")
                nc.vector.reciprocal_approx_fast(rb_r, rb_t)
                for h in (0, 1):
                    dst = aT[b][h * 64:(h + 1) * 64, qc * 512:(qc + 1) * 512]
                    nc.vector.tensor_tensor(dst, pvcs[h][0:64, :],
                                            rb_r[:, h, :], op=MULT)
                    if use_vbias:
                        nc.vector.tensor_scalar_add(
                            dst, dst, bv_sb[h * 64:(h + 1) * 64, 0:1]
                        )
            def emit_proj(b, qc):
                for nt in range(8):
                    ps = ps_a.tile([128, 512], f32, tag="mm")
                    nc.tensor.matmul(
                        ps,
                        wp_sb[:, nt * 128:(nt + 1) * 128],
                        aT[b][:, qc * 512:(qc + 1) * 512],
                        start=True, stop=True,
                    )
                    ob = obp.tile([128, 512], f32, tag="ob")
                    nc.vector.tensor_copy(ob, ps)
                    nc.sync.dma_start(
                        out_d[nt * 128:(nt + 1) * 128,
                              b * Tb + qc * 512: b * Tb + (qc + 1) * 512],
                        ob,
                    )

            # attention q-chunk qc only needs qkv chunks <= qc of its batch,
            # so attend as soon as each chunk's qT/kT/vb are ready; the
            # projection of chunk i-1 is emitted behind chunk i's whole
            # attention loop so the normalize chain's DMA-bounce latency
            # hides under attention matmuls instead of head-of-line-blocking
            # the PE queue
            prev = None
            for b in range(B):
                for mc in range(mc_per_b):
                    emit_qkv_chunk(b, mc)
                    emit_attn_qc(b, mc)
                    if prev is not None:
                        emit_norm(*prev)
                        emit_proj(*prev)
                    prev = (b, mc)
            emit_norm(*prev)
            emit_proj(*prev)

    nc.compile()
    return nc


def make_in_maps(x, w_qkv, b_qkv, use_vbias):
    """Host-side shard prep. Returns per-core input maps (w_proj added later)."""
    Mx = x.shape[0] * x.shape[1]
    xT = np.ascontiguousarray(x.reshape(Mx, D_MODEL).T)
    in_maps = []
    for c in range(N_CORES):
        w3c = np.ascontiguousarray(
            np.concatenate(
                [w_qkv[:, s * D_MODEL + c * RC: s * D_MODEL + (c + 1) * RC]
                 for s in range(3)],
                axis=1,
            )
        )
        bqkc = np.ascontiguousarray(
            np.stack(
                [b_qkv[c * RC:(c + 1) * RC],
                 b_qkv[D_MODEL + c * RC: D_MODEL + (c + 1) * RC]],
                axis=1,
            )
        )
        im = {"xT": xT, "w3": w3c, "bqk": bqkc,
              "ident": np.eye(128, dtype=np.float32)}
        if use_vbias:
            im["bv"] = np.ascontiguousarray(
                b_qkv[2 * D_MODEL + c * RC: 2 * D_MODEL + (c + 1) * RC][:, None]
            )
        in_maps.append(im)
    return in_maps


def kernel(x, w_qkv, b_qkv, w_proj, b_proj):
    from concourse.bass_utils import run_bass_kernel_spmd

    x = np.asarray(x, dtype=np.float32)
    w_qkv = np.asarray(w_qkv, dtype=np.float32)
    b_qkv = np.asarray(b_qkv, dtype=np.float32)
    w_proj = np.asarray(w_proj, dtype=np.float32)
    b_proj = np.asarray(b_proj, dtype=np.float32)

    use_vbias = bool(np.any(b_qkv[2 * D_MODEL:]))
    key = (T, use_vbias)
    if key not in _prog_cache:
        _prog_cache[key] = build_program(T, use_vbias)
    nc = _prog_cache[key]

    in_maps = make_in_maps(x, w_qkv, b_qkv, use_vbias)
    for c in range(N_CORES):
        in_maps[c]["wp"] = np.ascontiguousarray(w_proj[c * RC:(c + 1) * RC, :])

    res = run_bass_kernel_spmd(nc, in_maps, core_ids=list(range(N_CORES)))
    global _last_results
    _last_results = res
    total = res.results[0]["out"].copy()
    for c in range(1, N_CORES):
        total += res.results[c]["out"]
    out = total.T.reshape(B, T, D_MODEL) + b_proj[None, None, :]
    return np.ascontiguousarray(out.astype(np.float32))
